# revision 31
# baseline (speedup 1.0000x reference)
"""GQA attention kernel for 8 TRN2 NeuronCores.

Problem: B=2, T=2048, C=4096, NH=32 q-heads, NKV=8 kv-heads, HD=128,
RoPE (theta=1e4), causal, f32 I/O.

Sharding: core = (batch b, kv-head-group g): b = core//4, g = core%4.
Each core owns batch b and kv heads {2g, 2g+1} (= q heads 8g..8g+7):
  - projects x[b] against its wq/wk/wv column slices (bf16 compute),
  - runs causal attention for its 8 q heads,
  - computes the partial o_proj x its wo row slice -> [T, C] f32.
Host sums the 4 partials per batch.

On-chip layout is feature-major ("X^T"): activations live as
[feature=partition, token=free] so every matmul contracts along
partitions. x is pre-transposed/bf16-cast on host; RoPE's rotate_half
is a 128x128 permutation matmul on the PE; softmax denominator comes
free from a ones-column appended to V.
"""

import sys

sys.path.insert(0, "/opt/trn_rl_repo")

import numpy as np
import ml_dtypes

import concourse.bass as bass
import concourse.bacc as bacc
import concourse.mybir as mybir
import concourse.tile as tile
from concourse.bass_utils import run_bass_kernel_spmd

BF16 = mybir.dt.bfloat16
F32 = mybir.dt.float32
AF = mybir.ActivationFunctionType
ALU = mybir.AluOpType

B, T, C = 2, 2048, 4096
NH, NKV, HD = 32, 8, 128
THETA = 10000.0
NCORES = 8

QH = 8          # q heads per core
KV = 2          # kv heads per core
OUTS = 12       # projection output tiles: 8 q + 2 k + 2 v
QC = 4          # token chunks of 512
KT = 16         # k tiles of 128
TT = 16         # token tiles of 128
CCH = 32        # contraction chunks of 128 over C

_CACHE = {}


def _build_nc():
    nc = bacc.Bacc("TRN2", target_bir_lowering=False, debug=False,
                   enable_asserts=False, num_devices=NCORES)

    xT_d = nc.dram_tensor("xT", [C, T], BF16, kind="ExternalInput")
    wqkv_d = nc.dram_tensor("wqkv", [CCH, 2, 128, 768], BF16, kind="ExternalInput")
    wo_d = nc.dram_tensor("wo", [QH * HD, C], BF16, kind="ExternalInput")
    cos_d = nc.dram_tensor("cosT", [128, T], BF16, kind="ExternalInput")
    sin_d = nc.dram_tensor("sinT", [128, T], BF16, kind="ExternalInput")
    prot_d = nc.dram_tensor("protT", [128, 128], BF16, kind="ExternalInput")
    ident_d = nc.dram_tensor("ident", [128, 128], BF16, kind="ExternalInput")
    cmask_d = nc.dram_tensor("cmask", [128, 4, 512], F32, kind="ExternalInput")
    out_d = nc.dram_tensor("out", [T, C], F32, kind="ExternalOutput")

    with tile.TileContext(nc) as tc:
        with tc.tile_pool(name="persist", bufs=1) as pp:
            ident = pp.tile([128, 128], BF16)
            nc.sync.dma_start(ident, ident_d.ap())
            cosT = pp.tile([128, T], BF16)
            sinT = pp.tile([128, T], BF16)
            prot = pp.tile([128, 128], BF16)
            cmask = pp.tile([128, 4, 512], F32)

            # HAM warm-up: keep the PE busy while the first x^T block
            # DMAs in, so projections start at 2.4 GHz instead of 1.2.
            with tc.tile_pool(name="pwarm", bufs=2, space="PSUM") as pwp:
                for w in range(48):
                    wps = pwp.tile([128, 128], BF16, name=f"warm{w}", tag="warm")
                    nc.tensor.transpose(wps, ident, ident)

            QT = pp.tile([128, QH, T], BF16)
            KTt = pp.tile([128, KV, T], BF16)
            VT = pp.tile([128, KV, T], BF16)
            OT = pp.tile([128, QH, T], BF16)
            Vn = pp.tile([128, KV, KT, 132], BF16)
            nc.vector.memset(Vn[:, :, :, 128:129], 1.0)

            # ---------------- projections: Q^T/K^T/V^T = W^T @ x^T ----------
            with tc.tile_pool(name="xt", bufs=2) as xtp, \
                 tc.tile_pool(name="wt", bufs=6) as wtp, \
                 tc.tile_pool(name="pproj", bufs=7, space="PSUM") as ppj:
                xview = xT_d.ap().rearrange("(c p) t -> p c t", p=128)
                for qc in range(QC):
                    tsl = slice(qc * 512, (qc + 1) * 512)
                    xt = xtp.tile([128, CCH, 512], BF16)
                    # split the load (early c-chunks land first) and use the
                    # scalar HWDGE queue so weights stream in parallel on sync
                    for piece in range(4):
                        csl = slice(piece * 8, (piece + 1) * 8)
                        nc.scalar.dma_start(xt[:, csl, :], xview[:, csl, tsl])
                    for grp in range(2):
                        psums = [ppj.tile([128, 512], F32, name=f"pj{qc}_{grp}_{o}",
                                          tag="pj") for o in range(6)]
                        for c in range(CCH):
                            wt = wtp.tile([128, 768], BF16)
                            nc.sync.dma_start(wt, wqkv_d.ap()[c, grp])
                            for o in range(6):
                                nc.tensor.matmul(
                                    psums[o], wt[:, o * 128:(o + 1) * 128],
                                    xt[:, c, :], start=(c == 0), stop=(c == CCH - 1))
                        for o in range(6):
                            oi = grp * 6 + o
                            if oi < 8:
                                dst = QT[:, oi, tsl]
                            elif oi < 10:
                                dst = KTt[:, oi - 8, tsl]
                            else:
                                dst = VT[:, oi - 10, tsl]
                            # alternate engines so psum slots free faster
                            if o % 2 == 0:
                                nc.scalar.copy(dst, psums[o])
                            else:
                                nc.vector.tensor_copy(dst, psums[o])

            # constants for RoPE/attention — loaded once projections are
            # underway so they don't delay the first weight tiles
            nc.scalar.dma_start(cosT, cos_d.ap())
            nc.scalar.dma_start(sinT, sin_d.ap())
            nc.scalar.dma_start(prot, prot_d.ap())
            nc.scalar.dma_start(cmask, cmask_d.ap())

            # wo load after the x^T/weight stream pools are gone, so it
            # overlaps RoPE + attention without blowing SBUF
            wo_pool = tc.alloc_tile_pool(name="wop", bufs=1)
            wo_t = wo_pool.tile([128, QH, C], BF16)
            nc.sync.dma_start(wo_t, wo_d.ap().rearrange("(h p) n -> p h n", p=128))

            # ---------------- attention (with fused RoPE) ------------------
            # rot = P_rot @ q (sign baked into P_rot), q' = q*cos + rot*sin
            # S^T[k,q] = K @ Q^T; P^T = exp(S^T + mask); O = P @ [V|1]
            with tc.tile_pool(name="pst", bufs=3, space="PSUM") as stp, \
                 tc.tile_pool(name="po", bufs=5, space="PSUM") as pop, \
                 tc.tile_pool(name="pt", bufs=6) as ptp, \
                 tc.tile_pool(name="ob", bufs=4) as obp, \
                 tc.tile_pool(name="ropes", bufs=3) as rsp, \
                 tc.tile_pool(name="rc", bufs=4) as rcp:

                def rope(src):
                    for rqc in range(QC):
                        rsl = slice(rqc * 512, (rqc + 1) * 512)
                        ps = stp.tile([128, 512], F32, name=f"rot{rqc}", tag="st")
                        nc.tensor.matmul(ps, prot, src[:, rsl], start=True,
                                         stop=True)
                        rs = rsp.tile([128, 512], BF16, name=f"rs{rqc}", tag="rs")
                        nc.vector.tensor_tensor(rs, ps, sinT[:, rsl], op=ALU.mult)
                        nc.vector.tensor_tensor(src[:, rsl], src[:, rsl],
                                                cosT[:, rsl], op=ALU.mult)
                        nc.vector.tensor_tensor(src[:, rsl], src[:, rsl], rs,
                                                op=ALU.add)

                def vtrans(kv):
                    for kt in range(KT):
                        pt = stp.tile([128, 128], BF16, name=f"tv{kv}_{kt}",
                                      tag="st")
                        nc.tensor.transpose(
                            pt, VT[:, kv, kt * 128:(kt + 1) * 128], ident)
                        nc.vector.tensor_copy(Vn[:, kv, kt, 0:128], pt)

                rope(KTt[:, 0, :])
                vtrans(0)
                rope(QT[:, 0, :])
                rope(KTt[:, 1, :])
                vtrans(1)

                for h in range(QH):
                    kv = h // 4
                    if h + 1 < QH:
                        rope(QT[:, h + 1, :])
                    for qc in range(QC):
                        tsl = slice(qc * 512, (qc + 1) * 512)
                        po = [pop.tile([128, 129], F32, name=f"po{h}_{qc}_{j}",
                                       tag="po") for j in range(4)]
                        for kt in range(4 * qc + 4):
                            st = stp.tile([128, 512], F32, tag="st")
                            nc.tensor.matmul(
                                st, KTt[:, kv, kt * 128:(kt + 1) * 128],
                                QT[:, h, tsl], start=True, stop=True)
                            d = kt - 4 * qc
                            ptile = ptp.tile([128, 512], BF16)
                            if d >= 0:
                                # columns < d*128 are fully masked (skip);
                                # only the [d*128,(d+1)*128) block straddles
                                # the diagonal and needs the additive mask
                                bsl = slice(d * 128, (d + 1) * 128)
                                vsl = slice(d * 128, 512)
                                nc.vector.tensor_tensor(
                                    st[:, bsl], st[:, bsl], cmask[:, d, bsl],
                                    op=ALU.add)
                                nc.scalar.activation(ptile[:, vsl], st[:, vsl],
                                                     AF.Exp)
                            else:
                                nc.scalar.activation(ptile, st, AF.Exp)
                            for j in range(4):
                                qt = 4 * qc + j
                                if kt <= qt:
                                    nc.tensor.matmul(
                                        po[j], ptile[:, j * 128:(j + 1) * 128],
                                        Vn[:, kv, kt, 0:129],
                                        start=(kt == 0), stop=(kt == qt))
                        for j in range(4):
                            qt = 4 * qc + j
                            rc = rcp.tile([128, 1], F32)
                            nc.vector.reciprocal(rc, po[j][:, 128:129])
                            # store O natural [tok, hd] into OT's block; the
                            # in-place transpose batch below fixes the layout
                            # without stalling the PE mid-attention
                            nc.vector.tensor_scalar_mul(
                                OT[:, h, qt * 128:(qt + 1) * 128],
                                po[j][:, 0:128], rc)

            # ---------------- o_proj partial: O @ wo_slice ----------------
            with tc.tile_pool(name="pout", bufs=6, space="PSUM") as outp, \
                 tc.tile_pool(name="potr", bufs=2, space="PSUM") as otrp, \
                 tc.tile_pool(name="ostg", bufs=6) as stgp:
                # batched in-place transposes: OT blocks [tok,hd] -> [hd,tok]
                for h in range(QH):
                    for qt in range(TT):
                        osl = slice(qt * 128, (qt + 1) * 128)
                        ptr = otrp.tile([128, 128], BF16,
                                        name=f"otr{h}_{qt}", tag="otr")
                        nc.tensor.transpose(ptr, OT[:, h, osl], ident)
                        nc.vector.tensor_copy(OT[:, h, osl], ptr)
                for tt in range(TT):
                    psl = slice(tt * 128, (tt + 1) * 128)
                    for n in range(8):
                        nsl = slice(n * 512, (n + 1) * 512)
                        ps = outp.tile([128, 512], F32)
                        for h in range(QH):
                            nc.tensor.matmul(ps, OT[:, h, psl],
                                             wo_t[:, h, nsl],
                                             start=(h == 0), stop=(h == QH - 1))
                        stg = stgp.tile([128, 512], F32)
                        nc.scalar.copy(stg, ps)
                        nc.sync.dma_start(out_d.ap()[psl, nsl], stg)

            wo_pool.release()

    nc.compile()
    return nc


def _host_prep(x, wq, wk, wv, wo):
    bf = ml_dtypes.bfloat16
    scale = HD ** -0.5

    # RoPE tables, feature-major [128, T]
    inv_freq = 1.0 / (THETA ** (np.arange(0, HD, 2, dtype=np.float32) / HD))
    t = np.arange(T, dtype=np.float32)
    freqs = np.outer(t, inv_freq)                      # [T, 64]
    emb = np.concatenate([freqs, freqs], -1)           # [T, 128]
    cosT = np.ascontiguousarray(np.cos(emb).T).astype(bf)
    sinT = np.ascontiguousarray(np.sin(emb).T).astype(bf)

    # rotate_half as a permutation matrix, pre-transposed for lhsT:
    # rot = P_rot @ q with P_rot[i, i+64] = -1 (i<64), P_rot[i, i-64] = +1.
    protT = np.zeros((128, 128), np.float32)
    for i in range(64):
        protT[i + 64, i] = -1.0
        protT[i, i + 64] = 1.0
    protT = protT.astype(bf)

    ident = np.eye(128, dtype=np.float32).astype(bf)

    # additive causal masks for the 4 diagonal [128k, 512q] tiles
    # valid iff q_local >= d*128 + k_local
    kl = np.arange(128)[:, None]
    ql = np.arange(512)[None, :]
    cmask = np.stack(
        [np.where(ql >= d * 128 + kl, 0.0, -1e9).astype(np.float32)
         for d in range(4)], axis=1)                   # [128, 4, 512]
    cmask = np.ascontiguousarray(cmask)

    xT = []
    for b in range(B):
        xT.append(np.ascontiguousarray(x[b].astype(bf).T))

    wqkv, wob = [], []
    for g in range(4):
        q_s = (wq[:, g * 1024:(g + 1) * 1024] * scale).astype(bf)
        k_s = wk[:, g * 256:(g + 1) * 256].astype(bf)
        v_s = wv[:, g * 256:(g + 1) * 256].astype(bf)
        wall = np.concatenate([q_s, k_s, v_s], axis=1)       # [C, 1536]
        wall = wall.reshape(CCH, 128, 2, 768).transpose(0, 2, 1, 3)
        wqkv.append(np.ascontiguousarray(wall))              # [32, 2, 128, 768]
        wob.append(np.ascontiguousarray(
            wo[g * 1024:(g + 1) * 1024, :].astype(bf)))      # [1024, C]

    in_maps = []
    for core in range(NCORES):
        b, g = core // 4, core % 4
        in_maps.append({
            "xT": xT[b], "wqkv": wqkv[g], "wo": wob[g],
            "cosT": cosT, "sinT": sinT, "protT": protT,
            "ident": ident, "cmask": cmask,
        })
    return in_maps


def kernel(x, wq, wk, wv, wo, _trace=False, _tmpdir=None):
    if "nc" not in _CACHE:
        _CACHE["nc"] = _build_nc()
    nc = _CACHE["nc"]

    in_maps = _host_prep(x, wq, wk, wv, wo)
    res = run_bass_kernel_spmd(nc, in_maps, core_ids=list(range(NCORES)),
                               trace=_trace, tmpdir=_tmpdir)
    _CACHE["last_results"] = res

    out = np.zeros((B, T, C), np.float32)
    for core in range(NCORES):
        out[core // 4] += res.results[core]["out"]
    return out


# revision 33
# speedup vs baseline: 1.0175x; 1.0175x over previous
"""GQA attention kernel for 8 TRN2 NeuronCores.

Problem: B=2, T=2048, C=4096, NH=32 q-heads, NKV=8 kv-heads, HD=128,
RoPE (theta=1e4), causal, f32 I/O.

Sharding: core = (batch b, kv-head-group g): b = core//4, g = core%4.
Each core owns batch b and kv heads {2g, 2g+1} (= q heads 8g..8g+7):
  - projects x[b] against its wq/wk/wv column slices (bf16 compute),
  - runs causal attention for its 8 q heads,
  - computes the partial o_proj x its wo row slice -> [T, C] f32.
Host sums the 4 partials per batch.

On-chip layout is feature-major ("X^T"): activations live as
[feature=partition, token=free] so every matmul contracts along
partitions. x is pre-transposed/bf16-cast on host; RoPE's rotate_half
is a 128x128 permutation matmul on the PE; softmax denominator comes
free from a ones-column appended to V.
"""

import sys

sys.path.insert(0, "/opt/trn_rl_repo")

import numpy as np
import ml_dtypes

import concourse.bass as bass
import concourse.bacc as bacc
import concourse.mybir as mybir
import concourse.tile as tile
from concourse.bass_utils import run_bass_kernel_spmd

BF16 = mybir.dt.bfloat16
F32 = mybir.dt.float32
AF = mybir.ActivationFunctionType
ALU = mybir.AluOpType

B, T, C = 2, 2048, 4096
NH, NKV, HD = 32, 8, 128
THETA = 10000.0
NCORES = 8

QH = 8          # q heads per core
KV = 2          # kv heads per core
OUTS = 12       # projection output tiles: 8 q + 2 k + 2 v
QC = 4          # token chunks of 512
KT = 16         # k tiles of 128
TT = 16         # token tiles of 128
CCH = 32        # contraction chunks of 128 over C

_CACHE = {}


def _build_nc():
    nc = bacc.Bacc("TRN2", target_bir_lowering=False, debug=False,
                   enable_asserts=False, num_devices=NCORES)

    xT_d = nc.dram_tensor("xT", [C, T], BF16, kind="ExternalInput")
    wqkv_d = nc.dram_tensor("wqkv", [CCH, 2, 128, 768], BF16, kind="ExternalInput")
    wo_d = nc.dram_tensor("wo", [QH * HD, C], BF16, kind="ExternalInput")
    cos_d = nc.dram_tensor("cosT", [128, T], BF16, kind="ExternalInput")
    sin_d = nc.dram_tensor("sinT", [128, T], BF16, kind="ExternalInput")
    prot_d = nc.dram_tensor("protT", [128, 128], BF16, kind="ExternalInput")
    ident_d = nc.dram_tensor("ident", [128, 128], BF16, kind="ExternalInput")
    cmask_d = nc.dram_tensor("cmask", [128, 4, 512], F32, kind="ExternalInput")
    out_d = nc.dram_tensor("out", [T, C], F32, kind="ExternalOutput")

    with tile.TileContext(nc) as tc:
        with tc.tile_pool(name="persist", bufs=1) as pp:
            ident = pp.tile([128, 128], BF16)
            nc.sync.dma_start(ident, ident_d.ap())
            cosT = pp.tile([128, T], BF16)
            sinT = pp.tile([128, T], BF16)
            prot = pp.tile([128, 128], BF16)
            cmask = pp.tile([128, 4, 512], F32)

            # HAM warm-up: keep the PE busy while the first x^T block
            # DMAs in, so projections start at 2.4 GHz instead of 1.2.
            with tc.tile_pool(name="pwarm", bufs=2, space="PSUM") as pwp:
                for w in range(32):
                    wps = pwp.tile([128, 128], BF16, name=f"warm{w}", tag="warm")
                    nc.tensor.transpose(wps, ident, ident)

            QT = pp.tile([128, QH, T], BF16)
            KTt = pp.tile([128, KV, T], BF16)
            VT = pp.tile([128, KV, T], BF16)
            OT = pp.tile([128, QH, T], BF16)
            Vn = pp.tile([128, KV, KT, 132], BF16)
            nc.vector.memset(Vn[:, :, :, 128:129], 1.0)

            # ---------------- projections: Q^T/K^T/V^T = W^T @ x^T ----------
            with tc.tile_pool(name="xt", bufs=2) as xtp, \
                 tc.tile_pool(name="wt", bufs=6) as wtp, \
                 tc.tile_pool(name="pproj", bufs=7, space="PSUM") as ppj:
                xview = xT_d.ap().rearrange("(c p) t -> p c t", p=128)
                for qc in range(QC):
                    tsl = slice(qc * 512, (qc + 1) * 512)
                    xt = xtp.tile([128, CCH, 512], BF16)
                    # split the load (early c-chunks land first) and use the
                    # scalar HWDGE queue so weights stream in parallel on sync
                    for piece in range(4):
                        csl = slice(piece * 8, (piece + 1) * 8)
                        nc.scalar.dma_start(xt[:, csl, :], xview[:, csl, tsl])
                    for grp in range(2):
                        psums = [ppj.tile([128, 512], F32, name=f"pj{qc}_{grp}_{o}",
                                          tag="pj") for o in range(6)]
                        for c in range(CCH):
                            wt = wtp.tile([128, 768], BF16)
                            nc.sync.dma_start(wt, wqkv_d.ap()[c, grp])
                            for o in range(6):
                                nc.tensor.matmul(
                                    psums[o], wt[:, o * 128:(o + 1) * 128],
                                    xt[:, c, :], start=(c == 0), stop=(c == CCH - 1))
                        for o in range(6):
                            oi = grp * 6 + o
                            if oi < 8:
                                dst = QT[:, oi, tsl]
                            elif oi < 10:
                                dst = KTt[:, oi - 8, tsl]
                            else:
                                dst = VT[:, oi - 10, tsl]
                            # alternate engines so psum slots free faster
                            if o % 2 == 0:
                                nc.scalar.copy(dst, psums[o])
                            else:
                                nc.vector.tensor_copy(dst, psums[o])

            # constants for RoPE/attention — loaded once projections are
            # underway so they don't delay the first weight tiles
            nc.scalar.dma_start(cosT, cos_d.ap())
            nc.scalar.dma_start(sinT, sin_d.ap())
            nc.scalar.dma_start(prot, prot_d.ap())
            nc.scalar.dma_start(cmask, cmask_d.ap())

            # wo load after the x^T/weight stream pools are gone, so it
            # overlaps RoPE + attention without blowing SBUF
            wo_pool = tc.alloc_tile_pool(name="wop", bufs=1)
            wo_t = wo_pool.tile([128, QH, C], BF16)
            nc.sync.dma_start(wo_t, wo_d.ap().rearrange("(h p) n -> p h n", p=128))

            # ---------------- attention (with fused RoPE) ------------------
            # rot = P_rot @ q (sign baked into P_rot), q' = q*cos + rot*sin
            # S^T[k,q] = K @ Q^T; P^T = exp(S^T + mask); O = P @ [V|1]
            with tc.tile_pool(name="pst", bufs=4, space="PSUM") as stp, \
                 tc.tile_pool(name="po", bufs=4, space="PSUM") as pop, \
                 tc.tile_pool(name="pt", bufs=6) as ptp, \
                 tc.tile_pool(name="ob", bufs=4) as obp, \
                 tc.tile_pool(name="ropes", bufs=3) as rsp, \
                 tc.tile_pool(name="rc", bufs=4) as rcp:

                def rope(src):
                    for rqc in range(QC):
                        rsl = slice(rqc * 512, (rqc + 1) * 512)
                        ps = stp.tile([128, 512], F32, name=f"rot{rqc}", tag="st")
                        nc.tensor.matmul(ps, prot, src[:, rsl], start=True,
                                         stop=True)
                        rs = rsp.tile([128, 512], BF16, name=f"rs{rqc}", tag="rs")
                        nc.vector.tensor_tensor(rs, ps, sinT[:, rsl], op=ALU.mult)
                        nc.vector.tensor_tensor(src[:, rsl], src[:, rsl],
                                                cosT[:, rsl], op=ALU.mult)
                        nc.vector.tensor_tensor(src[:, rsl], src[:, rsl], rs,
                                                op=ALU.add)

                def vtrans(kv):
                    for kt in range(KT):
                        pt = stp.tile([128, 128], BF16, name=f"tv{kv}_{kt}",
                                      tag="st")
                        nc.tensor.transpose(
                            pt, VT[:, kv, kt * 128:(kt + 1) * 128], ident)
                        nc.vector.tensor_copy(Vn[:, kv, kt, 0:128], pt)

                rope(KTt[:, 0, :])
                vtrans(0)
                rope(QT[:, 0, :])
                rope(KTt[:, 1, :])
                vtrans(1)

                for h in range(QH):
                    kv = h // 4
                    if h + 1 < QH:
                        rope(QT[:, h + 1, :])
                    for qc in range(QC):
                        tsl = slice(qc * 512, (qc + 1) * 512)
                        po = [pop.tile([128, 129], F32, name=f"po{h}_{qc}_{j}",
                                       tag="po") for j in range(4)]
                        for kt in range(4 * qc + 4):
                            st = stp.tile([128, 512], F32, tag="st")
                            nc.tensor.matmul(
                                st, KTt[:, kv, kt * 128:(kt + 1) * 128],
                                QT[:, h, tsl], start=True, stop=True)
                            d = kt - 4 * qc
                            ptile = ptp.tile([128, 512], BF16)
                            if d >= 0:
                                # columns < d*128 are fully masked (skip);
                                # only the [d*128,(d+1)*128) block straddles
                                # the diagonal and needs the additive mask
                                bsl = slice(d * 128, (d + 1) * 128)
                                vsl = slice(d * 128, 512)
                                nc.vector.tensor_tensor(
                                    st[:, bsl], st[:, bsl], cmask[:, d, bsl],
                                    op=ALU.add)
                                nc.scalar.activation(ptile[:, vsl], st[:, vsl],
                                                     AF.Exp)
                            else:
                                nc.scalar.activation(ptile, st, AF.Exp)
                            for j in range(4):
                                qt = 4 * qc + j
                                if kt <= qt:
                                    nc.tensor.matmul(
                                        po[j], ptile[:, j * 128:(j + 1) * 128],
                                        Vn[:, kv, kt, 0:129],
                                        start=(kt == 0), stop=(kt == qt))
                        for j in range(4):
                            qt = 4 * qc + j
                            rc = rcp.tile([128, 1], F32)
                            nc.vector.reciprocal(rc, po[j][:, 128:129])
                            # store O natural [tok, hd] into OT's block; the
                            # in-place transpose batch below fixes the layout
                            # without stalling the PE mid-attention
                            nc.vector.tensor_scalar_mul(
                                OT[:, h, qt * 128:(qt + 1) * 128],
                                po[j][:, 0:128], rc)

            # ---------------- o_proj partial: O @ wo_slice ----------------
            with tc.tile_pool(name="pout", bufs=6, space="PSUM") as outp, \
                 tc.tile_pool(name="potr", bufs=2, space="PSUM") as otrp, \
                 tc.tile_pool(name="ostg", bufs=6) as stgp:
                # batched in-place transposes: OT blocks [tok,hd] -> [hd,tok]
                for h in range(QH):
                    for qt in range(TT):
                        osl = slice(qt * 128, (qt + 1) * 128)
                        ptr = otrp.tile([128, 128], BF16,
                                        name=f"otr{h}_{qt}", tag="otr")
                        nc.tensor.transpose(ptr, OT[:, h, osl], ident)
                        nc.vector.tensor_copy(OT[:, h, osl], ptr)
                for tt in range(TT):
                    psl = slice(tt * 128, (tt + 1) * 128)
                    for n in range(8):
                        nsl = slice(n * 512, (n + 1) * 512)
                        ps = outp.tile([128, 512], F32)
                        for h in range(QH):
                            nc.tensor.matmul(ps, OT[:, h, psl],
                                             wo_t[:, h, nsl],
                                             start=(h == 0), stop=(h == QH - 1))
                        stg = stgp.tile([128, 512], F32)
                        nc.scalar.copy(stg, ps)
                        nc.sync.dma_start(out_d.ap()[psl, nsl], stg)

            wo_pool.release()

    nc.compile()
    return nc


def _host_prep(x, wq, wk, wv, wo):
    bf = ml_dtypes.bfloat16
    scale = HD ** -0.5

    # RoPE tables, feature-major [128, T]
    inv_freq = 1.0 / (THETA ** (np.arange(0, HD, 2, dtype=np.float32) / HD))
    t = np.arange(T, dtype=np.float32)
    freqs = np.outer(t, inv_freq)                      # [T, 64]
    emb = np.concatenate([freqs, freqs], -1)           # [T, 128]
    cosT = np.ascontiguousarray(np.cos(emb).T).astype(bf)
    sinT = np.ascontiguousarray(np.sin(emb).T).astype(bf)

    # rotate_half as a permutation matrix, pre-transposed for lhsT:
    # rot = P_rot @ q with P_rot[i, i+64] = -1 (i<64), P_rot[i, i-64] = +1.
    protT = np.zeros((128, 128), np.float32)
    for i in range(64):
        protT[i + 64, i] = -1.0
        protT[i, i + 64] = 1.0
    protT = protT.astype(bf)

    ident = np.eye(128, dtype=np.float32).astype(bf)

    # additive causal masks for the 4 diagonal [128k, 512q] tiles
    # valid iff q_local >= d*128 + k_local
    kl = np.arange(128)[:, None]
    ql = np.arange(512)[None, :]
    cmask = np.stack(
        [np.where(ql >= d * 128 + kl, 0.0, -1e9).astype(np.float32)
         for d in range(4)], axis=1)                   # [128, 4, 512]
    cmask = np.ascontiguousarray(cmask)

    xT = []
    for b in range(B):
        xT.append(np.ascontiguousarray(x[b].astype(bf).T))

    wqkv, wob = [], []
    for g in range(4):
        q_s = (wq[:, g * 1024:(g + 1) * 1024] * scale).astype(bf)
        k_s = wk[:, g * 256:(g + 1) * 256].astype(bf)
        v_s = wv[:, g * 256:(g + 1) * 256].astype(bf)
        wall = np.concatenate([q_s, k_s, v_s], axis=1)       # [C, 1536]
        wall = wall.reshape(CCH, 128, 2, 768).transpose(0, 2, 1, 3)
        wqkv.append(np.ascontiguousarray(wall))              # [32, 2, 128, 768]
        wob.append(np.ascontiguousarray(
            wo[g * 1024:(g + 1) * 1024, :].astype(bf)))      # [1024, C]

    in_maps = []
    for core in range(NCORES):
        b, g = core // 4, core % 4
        in_maps.append({
            "xT": xT[b], "wqkv": wqkv[g], "wo": wob[g],
            "cosT": cosT, "sinT": sinT, "protT": protT,
            "ident": ident, "cmask": cmask,
        })
    return in_maps


def kernel(x, wq, wk, wv, wo, _trace=False, _tmpdir=None):
    if "nc" not in _CACHE:
        _CACHE["nc"] = _build_nc()
    nc = _CACHE["nc"]

    in_maps = _host_prep(x, wq, wk, wv, wo)
    res = run_bass_kernel_spmd(nc, in_maps, core_ids=list(range(NCORES)),
                               trace=_trace, tmpdir=_tmpdir)
    _CACHE["last_results"] = res

    out = np.zeros((B, T, C), np.float32)
    for core in range(NCORES):
        out[core // 4] += res.results[core]["out"]
    return out
